# revision 1
# baseline (speedup 1.0000x reference)
"""Trainium2 Bass kernel for nn_CPSN (retrieval_knn PSM/PWG module).

Contract: kernel(**inputs) takes the FULL unsharded inputs (as produced by
setup_inputs) and returns the FULL output [2, b*q, s], distributing work
across 8 NeuronCores internally (data-parallel over the query dim q).

v2 changes vs the fp32 baseline:
  - matmul inputs are fp16 (normalized features + squared chunks): the PE
    streams 1 row/cycle instead of fp32's 4 cycles/row => ~4x less PE busy.
    PSUM accumulation stays fp32; measured end-to-end rel err ~4e-3 (<2e-2).
  - single resident block: fp16 halves SBUF so all 25 f1n images fit at
    once; normalization runs once, interleaved with the T phase.
  - normalize uses Act Sqrt(nsq+eps) + DVE reciprocal: near-zero DVE work.
  - v3: each PSUM similarity tile is copied to SBUF (fp32) by the Scalar
    engine right after the matmuls; the DVE reduce+stt then run out of SBUF,
    which enables the TensorScalarPtr 2x_2p fast mode (all-SBUF operands)
    and frees PSUM banks earlier for the next matmul group.
"""

import os
import sys

import numpy as np

for _p in ("/opt/trn_rl_repo", "/root/.axon_site/_ro/trn_rl_repo"):
    if os.path.isdir(_p) and _p not in sys.path:
        sys.path.insert(0, _p)

import concourse.bass as bass
import concourse.tile as tile
from concourse import bacc, library_config, mybir
from concourse.bass_utils import run_bass_kernel_spmd

# ---- problem constants (hardcoded per contract) ----
B, S, Q, C, H, W, TEMP = 1, 25, 30, 512, 19, 19, 64
HW = H * W  # 361
NCORES = 8
L = 4               # local (padded) query images per core; Q_PAD = 32
Q_PAD = NCORES * L
CCH = C // 128      # 4 contraction chunks
PCH = [(0, 128), (128, 128), (256, HW - 256)]  # pixel-dim partition chunks
GRP = 4             # O-phase ss group size (PSUM bank budget)
BN_EPS = 1e-5

F32 = mybir.dt.float32
F16 = mybir.dt.float16
AX_X = mybir.AxisListType.X
OP = mybir.AluOpType
AF = mybir.ActivationFunctionType


def _col_off(l, kind, pch, ss):
    # cols2d free layout: [L][kind:4][pchunk:3][S]
    return ((l * 4 + kind) * 3 + pch) * S + ss


def build_program(variant="", repeat=1):
    """Build the (SPMD-shared) single-core bass program."""
    nc = bacc.Bacc(None, target_bir_lowering=False, debug=False)

    f1_d = nc.dram_tensor("f1", [S, C, HW], F32, kind="ExternalInput")
    f2_d = nc.dram_tensor("f2s", [L, C, HW], F32, kind="ExternalInput")
    # attention rows, host-gathered per (l, ss); broadcast to 128 rows on-device
    a1r_d = nc.dram_tensor("a1r", [L, S, HW], F32, kind="ExternalInput")
    a2r_d = nc.dram_tensor("a2r", [L, HW], F32, kind="ExternalInput")
    out_d = nc.dram_tensor("out", [2 * L, S], F32, kind="ExternalOutput")

    with tile.TileContext(nc) as tc:
        from contextlib import ExitStack

        with ExitStack() as ctx:
            pp = ctx.enter_context(tc.tile_pool(name="pp", bufs=2, space="PSUM"))
            f1n_pool = ctx.enter_context(tc.tile_pool(name="f1n", bufs=S * CCH))
            f2n_pool = ctx.enter_context(tc.tile_pool(name="f2n", bufs=L * CCH))
            raw_pool = ctx.enter_context(tc.tile_pool(name="raw", bufs=12))
            sq_pool = ctx.enter_context(tc.tile_pool(name="sq", bufs=4))
            row_pool = ctx.enter_context(tc.tile_pool(name="rows", bufs=6))
            invbc_pool = ctx.enter_context(tc.tile_pool(name="invbc", bufs=3))
            a1bc_pool = ctx.enter_context(tc.tile_pool(name="a1bc", bufs=30))
            a2bc_pool = ctx.enter_context(tc.tile_pool(name="a2bc", bufs=L))
            stt_pool = ctx.enter_context(tc.tile_pool(name="sttscr", bufs=2))
            cp_pool = ctx.enter_context(tc.tile_pool(name="cp", bufs=4))
            cols_pool = ctx.enter_context(tc.tile_pool(name="cols", bufs=1))
            cst_pool = ctx.enter_context(tc.tile_pool(name="cst", bufs=3))
            fin_pool = ctx.enter_context(tc.tile_pool(name="fin", bufs=6))

            nc.gpsimd.load_library(library_config.lib)

            onescol = cst_pool.tile([128, CCH], F16, tag="cst")
            nc.vector.memset(onescol[:], 1.0)
            mcol = cst_pool.tile([128, 1], F32, tag="cst")
            nc.vector.memset(mcol[:], 1.0 / HW)
            epscol = cst_pool.tile([1, 1], F32, tag="cst")
            nc.vector.memset(epscol[:], 1e-20)

            def bcast_row(src_dram_ap, name):
                stg = row_pool.tile([1, HW], F32, name=f"stg_{name}", tag="rstg")
                nc.sync.dma_start(stg[:], src_dram_ap)
                t = a1bc_pool.tile([128, HW], F32, name=f"bc_{name}", tag="a1bc")
                nc.gpsimd.partition_broadcast(t[:], stg[0:1, :], channels=128)
                return t

            cols2d = cols_pool.tile([128, L * 4 * 3 * S], F32)
            cols12 = cols_pool.tile([128, 3 * S * L], F32)
            if variant:
                nc.vector.memset(cols2d[:], 1.0)
                nc.vector.memset(cols12[:], 1.0)

            # a2 broadcast tiles (persist whole kernel)
            a2bc = []
            for l in range(L):
                stg = row_pool.tile([1, HW], F32, name=f"stga2_{l}", tag="rstg")
                nc.sync.dma_start(stg[:], a2r_d[l:l + 1, :])
                t = a2bc_pool.tile([128, HW], F32, name=f"a2bc{l}", tag="a2bc")
                nc.gpsimd.partition_broadcast(t[:], stg[0:1, :], channels=128)
                a2bc.append(t)

            f1nt = {}
            f2nt = [[None] * CCH for _ in range(L)]

            def normalize_image(dst_tiles, src_ap_of_c):
                """DMA raw chunks, inv-norm per pixel via Rsqrt(sumsq+eps),
                write normalized fp16 chunks into dst_tiles."""
                raws = []
                nsq = pp.tile([1, 512], F32, tag="ps")
                for c in range(CCH):
                    rt = raw_pool.tile([128, HW], F32, name=f"raw{c}", tag="raw")
                    nc.sync.dma_start(rt[:], src_ap_of_c(c))
                    raws.append(rt)
                if "nonorm" in variant:
                    for c in range(CCH):
                        nc.scalar.activation(dst_tiles[c][:], raws[c][:], AF.Copy)
                    return
                for c in range(CCH):
                    sq = sq_pool.tile([128, HW], F16, name=f"sq{c}", tag="sq")
                    nc.scalar.activation(sq[:], raws[c][:], AF.Square)
                    nc.tensor.matmul(nsq[:, 0:HW], onescol[:, c:c + 1],
                                     sq[:],
                                     start=(c == 0), stop=(c == CCH - 1))
                # inv = 1/sqrt(sumsq + tiny): tiny keeps padded zero images
                # finite (they produce garbage that the host discards).
                nrm = row_pool.tile([1, HW], F32, tag="rows")
                nc.scalar.activation(nrm[:], nsq[0:1, 0:HW], AF.Sqrt,
                                     bias=epscol[0:1, 0:1])
                inv = row_pool.tile([1, HW], F32, tag="rows")
                nc.vector.reciprocal(inv[:], nrm[:])
                ibc = invbc_pool.tile([128, HW], F32, tag="invbc")
                nc.gpsimd.partition_broadcast(ibc[:], inv[0:1, :], channels=128)
                for c in range(CCH):
                    nc.gpsimd.tensor_tensor(dst_tiles[c][:], raws[c][:], ibc[:],
                                            op=OP.mult)

            for _rep in range(repeat):
                # ---- f2 normalization (needed by every T-phase tile) ----
                for l in range(L):
                    dst = [f2n_pool.tile([128, HW], F16, name=f"f2n_{l}_{c}",
                                         tag="f2n") for c in range(CCH)]
                    normalize_image(
                        dst, lambda c, l=l: f2_d[l, c * 128:(c + 1) * 128, :])
                    f2nt[l] = dst

                # ---- T phase, interleaved with f1 normalization ----
                # psT[y, l, x] per (ss, pch); weights = f1n chunks
                for ss in range(S):
                    dst = [f1n_pool.tile([128, HW], F16, name=f"f1n_{ss}_{c}",
                                         tag="f1n") for c in range(CCH)]
                    normalize_image(
                        dst, lambda c, ss=ss: f1_d[ss, c * 128:(c + 1) * 128, :])
                    f1nt[ss] = dst

                    for pi, (y0, yp) in enumerate(PCH):
                        psT = pp.tile([128, L, 512], F32, name="psT", tag="ps")
                        if "nomm" not in variant:
                            for l in range(L):
                                for c in range(CCH):
                                    nc.tensor.matmul(
                                        psT[0:yp, l, 0:HW],
                                        f1nt[ss][c][:, y0:y0 + yp],
                                        f2nt[l][c][:, :],
                                        start=(c == 0), stop=(c == CCH - 1))
                        else:
                            nc.vector.memset(psT[:, :, :], 0.1)
                        if "nodve" in variant:
                            continue
                        cp = cp_pool.tile([128, L, HW], F32, name="cpT",
                                          tag="cp")
                        nc.scalar.activation(cp[0:yp, :, :],
                                             psT[0:yp, :, 0:HW], AF.Copy)
                        o12 = (pi * S + ss) * L
                        nc.vector.reduce_max(cols12[0:yp, o12:o12 + L],
                                             psT[0:yp, :, 0:HW], axis=AX_X)
                        for l in range(L):
                            og = _col_off(l, 3, pi, ss)
                            scr = stt_pool.tile([128, HW], F16, name="sttscr",
                                                tag="sttscr")
                            nc.vector.scalar_tensor_tensor(
                                scr[0:yp, :], cp[0:yp, l, :],
                                cols12[0:yp, o12 + l:o12 + l + 1],
                                a2bc[l][0:yp, :],
                                op0=OP.is_ge, op1=OP.mult,
                                accum_out=cols2d[0:yp, og:og + 1])

                # ---- O phase: O[x, y] per (l, ss); weights = f2n chunks ----
                for l in range(L):
                    a1t = {}
                    for ss in range(S):
                        a1t[ss] = bcast_row(a1r_d[l, ss:ss + 1, :], f"{l}_{ss}")
                    for pi, (x0, xp) in enumerate(PCH):
                        for g0 in range(0, S, GRP):
                            grp = list(range(g0, min(g0 + GRP, S)))
                            ng = len(grp)
                            psO = pp.tile([128, L, 512], F32, name="psO",
                                          tag="ps")
                            if "nomm" not in variant:
                                for j, ss in enumerate(grp):
                                    for c in range(CCH):
                                        nc.tensor.matmul(
                                            psO[0:xp, j, 0:HW],
                                            f2nt[l][c][:, x0:x0 + xp],
                                            f1nt[ss][c][:, :],
                                            start=(c == 0), stop=(c == CCH - 1))
                            else:
                                nc.vector.memset(psO[:, :, :], 0.1)
                            if "nodve" in variant:
                                continue
                            cp = cp_pool.tile([128, L, HW], F32, name="cpO",
                                              tag="cp")
                            nc.scalar.activation(cp[0:xp, 0:ng, :],
                                                 psO[0:xp, 0:ng, 0:HW],
                                                 AF.Copy)
                            # s21 for the ng consecutive ss: contiguous cols
                            ob = _col_off(l, 0, pi, grp[0])
                            nc.vector.reduce_max(cols2d[0:xp, ob:ob + ng],
                                                 psO[0:xp, 0:ng, 0:HW],
                                                 axis=AX_X)
                            for j, ss in enumerate(grp):
                                og = _col_off(l, 2, pi, ss)
                                scr = stt_pool.tile([128, HW], F16,
                                                    name="sttscr", tag="sttscr")
                                nc.vector.scalar_tensor_tensor(
                                    scr[0:xp, :], cp[0:xp, j, :],
                                    cols2d[0:xp, ob + j:ob + j + 1],
                                    a1t[ss][0:xp, :],
                                    op0=OP.is_ge, op1=OP.mult,
                                    accum_out=cols2d[0:xp, og:og + 1])

                # ---- finals: w = g1*g2; out0 = mean(s12*w); out1 = mean(s21*w)
                for l in range(L):
                    fp1 = pp.tile([1, S], F32, tag="ps")
                    fp2 = pp.tile([1, S], F32, tag="ps")
                    for pi, (p0, pn) in enumerate(PCH):
                        g1 = cols2d[0:pn, _col_off(l, 2, pi, 0):_col_off(l, 2, pi, 0) + S]
                        g2 = cols2d[0:pn, _col_off(l, 3, pi, 0):_col_off(l, 3, pi, 0) + S]
                        s21 = cols2d[0:pn, _col_off(l, 0, pi, 0):_col_off(l, 0, pi, 0) + S]
                        c12 = cols12[0:pn, :]
                        s12 = bass.AP(c12.tensor, c12.offset + pi * S * L + l,
                                      [c12.ap[0], [L, S]])
                        wt = fin_pool.tile([128, S], F32, tag="fin")
                        v1 = fin_pool.tile([128, S], F32, tag="fin")
                        v2 = fin_pool.tile([128, S], F32, tag="fin")
                        nc.vector.tensor_mul(wt[0:pn, :], g1, g2)
                        nc.vector.tensor_mul(v1[0:pn, :], s12, wt[0:pn, :])
                        nc.vector.tensor_mul(v2[0:pn, :], s21, wt[0:pn, :])
                        nc.tensor.matmul(fp1[:, :], mcol[0:pn, 0:1], v1[0:pn, :],
                                         start=(pi == 0), stop=(pi == 2))
                        nc.tensor.matmul(fp2[:, :], mcol[0:pn, 0:1], v2[0:pn, :],
                                         start=(pi == 0), stop=(pi == 2))
                    st1 = fin_pool.tile([1, S], F32, name=f"st1_{l}", tag="finst")
                    st2 = fin_pool.tile([1, S], F32, name=f"st2_{l}", tag="finst")
                    nc.scalar.activation(st1[:], fp1[0:1, :], AF.Copy)
                    nc.scalar.activation(st2[:], fp2[0:1, :], AF.Copy)
                    nc.sync.dma_start(out_d[l:l + 1, :], st1[0:1, :])
                    nc.sync.dma_start(out_d[L + l:L + l + 1, :], st2[0:1, :])

    nc.finalize()
    return nc


def _meta_learner_host(x, W1, g1, b1, m1, v1, W2, g2, b2, m2, v2):
    """x: [N, C, HW] -> [N, HW]  (two 1x1 convs + eval BN + ReLU on host)."""
    inv1 = g1 / np.sqrt(v1 + BN_EPS)
    bias1 = b1 - m1 * inv1
    y = np.einsum("tc,ncp->ntp", W1, x, dtype=np.float32)
    y = np.maximum(y * inv1[None, :, None] + bias1[None, :, None], 0.0)
    inv2 = g2 / np.sqrt(v2 + BN_EPS)
    bias2 = b2 - m2 * inv2
    z = np.einsum("ot,ntp->nop", W2, y, dtype=np.float32)
    z = np.maximum(z * inv2[None, :, None] + bias2[None, :, None], 0.0)
    return z[:, 0, :]


_NC_CACHE = [None]


def _prepare_in_maps(f1, f2, W1, g1, b1, m1, v1, W2, g2, b2, m2, v2):
    f1 = np.asarray(f1, np.float32).reshape(S, C, HW)
    f2 = np.asarray(f2, np.float32).reshape(Q, C, HW)
    W1 = np.asarray(W1, np.float32)
    W2 = np.asarray(W2, np.float32)
    g1, b1, m1, v1 = (np.asarray(a, np.float32) for a in (g1, b1, m1, v1))
    g2, b2, m2, v2 = (np.asarray(a, np.float32) for a in (g2, b2, m2, v2))

    # host meta-learner (tiny): a1 [S, HW], a2 [Q, HW]
    a1 = _meta_learner_host(f1, W1, g1, b1, m1, v1, W2, g2, b2, m2, v2)
    a2 = _meta_learner_host(f2, W1, g1, b1, m1, v1, W2, g2, b2, m2, v2)

    f2p = np.zeros((Q_PAD, C, HW), np.float32)
    f2p[:Q] = f2
    a2p = np.zeros((Q_PAD, HW), np.float32)
    a2p[:Q] = a2

    in_maps = []
    for core in range(NCORES):
        qq = [core * L + l for l in range(L)]
        a1r = np.zeros((L, S, HW), np.float32)
        a2r = np.zeros((L, HW), np.float32)
        for l, q in enumerate(qq):
            if q < Q:
                for ss in range(S):
                    i1 = (q * S + ss) // Q  # faithful torch-layout quirk
                    a1r[l, ss] = a1[i1]
                a2r[l] = a2p[q]
        in_maps.append({
            "f1": f1,
            "f2s": f2p[core * L:(core + 1) * L],
            "a1r": a1r,
            "a2r": a2r,
        })

    return in_maps


def _assemble(res):
    s1 = np.zeros((Q, S), np.float32)
    s2 = np.zeros((Q, S), np.float32)
    for core in range(NCORES):
        o = res.results[core]["out"].reshape(2, L, S)
        for l in range(L):
            q = core * L + l
            if q < Q:
                s1[q] = o[0, l]
                s2[q] = o[1, l]
    return np.stack([s1, s2])


def kernel(**inputs):
    in_maps = _prepare_in_maps(**inputs)
    if _NC_CACHE[0] is None:
        _NC_CACHE[0] = build_program()
    res = run_bass_kernel_spmd(_NC_CACHE[0], in_maps, list(range(NCORES)))
    return _assemble(res)

